# revision 1
# baseline (speedup 1.0000x reference)
"""Additive attention (Bahdanau-style) TRN2 Bass kernel, SPMD over 8 NeuronCores.

Reference computation (B=4, Lq=Lk=512, D=H=128):
    q = queries @ Wq                     (B, Lq, H)
    k = keys @ Wk                        (B, Lk, H)
    scores[b,i,j] = sum_h wv[h] * tanh(q[b,i,h] + k[b,j,h])
    scores masked to -1e6 for j >= valid_seq_len[b] -> softmax over j -> @ values @ Wo

Sharding: data-parallel over Lq (each core takes 64 queries of EVERY batch,
so the per-core work is Sum_b valid_b * 64 regardless of the mask skew).
The kernel is specialized at build time to the actual valid_seq_len values
(masked key columns are simply never computed; exp() of a masked column is
exactly 0 in the reference because exp(-1e6 - max) underflows, so skipping
them is exact).

Per-core device program (h lives on SBUF partitions):
  qfT (h,i) = Wq^T @ qT,  kfT_b (h,j) = Wk^T @ kT_b          [PE]
  S chunk (h, G, V) = kfT broadcast + qfT broadcast          [DVE, stride-0 APs]
  F = tanh(S)                                                [ACT]
  scores rows: M=32 matmuls with a shifted-diagonal wv matrix Z so query r
    lands on PSUM partition r (accumulating +0 rows elsewhere)  [PE]
  softmax: reduce_max(negate) -> Exp(bias=-max, accum_out=rowsum)  [DVE+ACT]
  attn^T via PE transpose; PV accumulated over j-tiles; out = (pvT)^T@Wo,
  with the 1/rowsum folded into the final PSUM->SBUF copy as a per-row scale.
"""

import math
from contextlib import ExitStack

import numpy as np

B, LQ, LK, D, H = 4, 512, 512, 128, 128
NCORES = 8
QPC = LQ // NCORES  # queries per core per batch = 64
G = 16  # queries per DVE/ACT chunk
ORDER_MODE = "small_first"
ADD_MODE = "tt"  # "tt" = chunk tensor_tensor, "ts" = per-query tensor_scalar (2x_2P)
PAIR_OUT = False  # pair-wide output stage (fewer ops)
TUNE_BUFS = False  # asymmetric S/F pool split + deeper E pool
POOL_ALLOC = "queue"  # slot allocator: queue beat stack ~1.2us in both quiet-window A/B pairs
DVE_NS = 1.14  # measured ns/col for DVE broadcast add (unsliced)
VMAX = 512  # target columns per v-slice (512 = no slicing; DVE hates short rows)
POOL_NS = 1e9  # GPSIMD disabled: DVE TT locks the shared SBUF port pair, so GPSIMD cannot overlap

_RUNNERS: dict = {}


def _emit_body(nc, tc, ctx, consts, loads, valid, njs, dram, f32, fp16, AF, AX, variant="full", stage_cb=None):
    """One full attention pass. Safe to emit inside a For_i (idempotent)."""
    qT_d, kT_d, vals_d, wq_d, wk_d, wo_d, zmat_d, ident_d, out_d = dram

    desc = sorted(range(B), key=lambda b: -valid[b])
    if ORDER_MODE == "small_first":
        order = [desc[-1]] + desc[:-1]  # smallest first (fast fill), then big
    else:  # second-smallest first, big middle, smallest last (short drain)
        order = [desc[-2]] + desc[:-2] + [desc[-1]]

    wq_sb = loads.tile([D, H], f32, tag="wq")
    nc.sync.dma_start(wq_sb[:], wq_d[:])
    qT_sb = loads.tile([D, B * QPC], f32, tag="qT")
    nc.sync.dma_start(qT_sb[:], qT_d[:])
    wk_sb = loads.tile([D, H], f32, tag="wk")
    nc.sync.dma_start(wk_sb[:], wk_d[:])
    zmat_sb = loads.tile([H, 63], fp16, tag="zmat")
    nc.sync.dma_start(zmat_sb[:], zmat_d[:])

    kT_sb = {}
    for b in order:  # first-processed batch's keys first
        t = loads.tile([D, valid[b]], f32, name=f"kT{b}", tag=f"kT{b}")
        nc.sync.dma_start(t[:], kT_d[b * D : (b + 1) * D, 0 : valid[b]])
        kT_sb[b] = t

    # after the critical-path loads on the same HWDGE queue
    wo_sb = loads.tile([D, H], f32, tag="wo")
    nc.sync.dma_start(wo_sb[:], wo_d[:])
    ident_sb = loads.tile([128, 128], f32, tag="ident")
    nc.sync.dma_start(ident_sb[:], ident_d[:])
    vals_sb = {}
    for b in order:
        for jt in range(njs[b]):
            t = loads.tile([128, D], f32, name=f"vals{b}_{jt}", tag=f"vals{b}_{jt}")
            r0 = b * LK + jt * 128
            nc.sync.dma_start(t[:], vals_d[r0 : r0 + 128, :])
            vals_sb[(b, jt)] = t

    # ---- projections: qfT (h, B*QPC), kfT_b (h, LK) ----
    qfT_sb = consts.tile([H, B * QPC], f32, tag="qfT")
    kfT_sb = {}
    with tc.tile_pool(name="proj_ps", bufs=1, space="PSUM") as proj_ps:
        qf_ps = proj_ps.tile([H, B * QPC], f32, tag="qf")
        nc.tensor.matmul(qf_ps[:], lhsT=wq_sb[:], rhs=qT_sb[:], start=True, stop=True)
        nc.scalar.copy(qfT_sb[:], qf_ps[:])
        for b in order:
            V = valid[b]
            Ve = V + (V & 1)  # even width for DVE 2x_2P tensor_scalar mode
            kf_ps = proj_ps.tile([H, V], f32, name=f"kf{b}", tag=f"kf{b}")
            nc.tensor.matmul(
                kf_ps[:], lhsT=wk_sb[:], rhs=kT_sb[b][:], start=True, stop=True
            )
            t = consts.tile([H, Ve], f32, name=f"kfT{b}", tag=f"kfT{b}")
            nc.scalar.copy(t[:, 0:V], kf_ps[:])
            if Ve != V and ADD_MODE == "ts":
                nc.vector.memset(t[:, V:Ve], 0.0)
            kfT_sb[b] = t

    # S/F slot = largest chunk (G x maxV fp16); size pool depths to the SBUF left
    slot_kb = G * min(max(valid), VMAX) * 2 / 1024.0
    fixed_kb = (
        2 * (4.2 + sum(valid) * 4 / 1024.0)  # double-buffered input loads
        + 1.0 + sum(valid) * 4 / 1024.0      # qfT + kfT
        + 11.0                               # E tiles, stats, attnT, osb, slack
    )
    nslots = int((196.0 - fixed_kb) / slot_kb)
    if TUNE_BUFS:
        # S lives through the whole DVE->ACT handoff; F is consumed fast by PE
        bufs_s = max(3, min(10, nslots * 2 // 3))
        bufs_f = max(3, min(8, nslots - bufs_s))
    else:
        bufs_s = max(3, min(8, (nslots + 1) // 2))
        bufs_f = max(3, min(8, nslots - bufs_s))
    spool = ctx.enter_context(tc.tile_pool(name="s", bufs=bufs_s))
    fpool = ctx.enter_context(tc.tile_pool(name="f", bufs=bufs_f))
    scpool = ctx.enter_context(tc.tile_pool(name="scores", bufs=1, space="PSUM"))
    epool = ctx.enter_context(tc.tile_pool(name="e", bufs=3 if TUNE_BUFS else 2))
    stat = ctx.enter_context(tc.tile_pool(name="stat", bufs=8))
    tpool = ctx.enter_context(tc.tile_pool(name="attnT", bufs=4))
    tps = ctx.enter_context(tc.tile_pool(name="tps", bufs=2, space="PSUM"))
    pvps = ctx.enter_context(tc.tile_pool(name="pvps", bufs=1, space="PSUM"))
    opool = ctx.enter_context(tc.tile_pool(name="osb", bufs=2))

    pv_ps = pvps.tile([D, B * QPC], f32, tag="pv")
    rinvs = {}
    add_cost = {"dve": 0.0, "pool": 0.0}  # projected engine-finish (ns)

    warm = stat.tile([1, 1], f32, tag="warm")
    nc.vector.memset(warm[:], 0.0)
    warm2 = stat.tile([1, 1], f32, tag="warm2")
    nc.scalar.activation(warm2[:], warm[:], AF.Tanh)

    def vslices(V):
        n = max(1, -(-V // VMAX))
        base, rem = divmod(V, n)
        out, v0 = [], 0
        for i in range(n):
            vsz = base + (1 if i < rem else 0)
            out.append((v0, vsz))
            v0 += vsz
        return out

    def emit_bias_queries(b, sc_ps, q0, n):
        """Fused add+tanh on ACT via per-partition bias (no DVE work)."""
        V = valid[b]
        kin = kfT_sb[b][:, 0:V].unsqueeze(1)
        for i in range(q0, q0 + n):
            F1 = fpool.tile([H, 1, V], fp16, tag="f")
            nc.scalar.activation(
                F1[:], kin, AF.Tanh, bias=qfT_sb[:, b * QPC + i : b * QPC + i + 1]
            )
            grp, row = divmod(i, 32)
            nc.tensor.matmul(
                sc_ps[32 * grp : 32 * grp + 32, 0:V],
                lhsT=zmat_sb[:, 31 - row : 63 - row],
                rhs=F1[:, 0, :],
                start=(row == 0),
                stop=(row == 31),
            )

    def emit_chunks(b, sc_ps, last=False, nbias=0):
        V = valid[b]
        nq = QPC - nbias
        gslices = [(g * G, min(G, nq - g * G)) for g in range(-(-nq // G))]
        for q0, gsz in gslices:
            for v0, vsz in vslices(V):
                Spad = vsz + (vsz & 1) if ADD_MODE == "ts" else vsz
                S = spool.tile([H, gsz, Spad], fp16, tag="s")
                if ADD_MODE == "ts":
                    # per-query single-src adds hit DVE's 2x_2P fp32 mode
                    for r in range(gsz):
                        nc.vector.tensor_scalar_add(
                            S[:, r, :],
                            kfT_sb[b][:, v0 : v0 + Spad],
                            qfT_sb[:, b * QPC + q0 + r : b * QPC + q0 + r + 1],
                        )
                else:
                    kb = (
                        kfT_sb[b][:, v0 : v0 + vsz]
                        .unsqueeze(1)
                        .broadcast_to([H, gsz, vsz])
                    )
                    qc = (
                        qfT_sb[:, b * QPC + q0 : b * QPC + q0 + gsz]
                        .unsqueeze(2)
                        .broadcast_to([H, gsz, vsz])
                    )
                    nc.vector.tensor_add(S[:], kb, qc)
                if variant.startswith("addonly"):
                    continue
                F = fpool.tile([H, gsz, Spad], fp16, tag="f")
                nc.scalar.activation(F[:], S[:], AF.Tanh)
                if variant.startswith("notail"):
                    continue
                for r in range(gsz):
                    qi = q0 + r
                    grp, row = divmod(qi, 32)
                    nc.tensor.matmul(
                        sc_ps[32 * grp : 32 * grp + 32, v0 : v0 + vsz],
                        lhsT=zmat_sb[:, 31 - row : 63 - row],
                        rhs=F[:, r, 0:vsz],
                        start=(row == 0 and v0 == 0),
                        stop=(row == 31 and v0 + vsz == V),
                    )

    def emit_tail(b, sc_ps):
        """Softmax + attn^T + PV for batch b (emitted one batch late so the
        strict-FIFO ACT/DVE queues keep streaming the next batch's chunks)."""
        V = valid[b]
        nj = njs[b]
        negmax = stat.tile([QPC, 1], f32, tag="negmax")
        nc.vector.reduce_max(negmax[:], sc_ps[:, 0:V], axis=AX.X, negate=True)
        E = epool.tile([QPC, 512], f32, tag="e")
        if V < nj * 128:
            nc.vector.memset(E[:, V : nj * 128], 0.0)
        rowsum = stat.tile([QPC, 1], f32, tag="rowsum")
        nc.scalar.activation(
            E[:, 0:V], sc_ps[:, 0:V], AF.Exp, bias=negmax[:], accum_out=rowsum[:]
        )
        if not PAIR_OUT:
            rinv = stat.tile([QPC, 1], f32, tag=f"rinv{b}")
            nc.vector.reciprocal(rinv[:], rowsum[:])
            rinvs[b] = rinv

        for jt in range(nj):
            at_ps = tps.tile([128, QPC], f32, tag="atps")
            nc.tensor.transpose(
                at_ps[:], E[:, 128 * jt : 128 * (jt + 1)], ident_sb[0:QPC, 0:QPC]
            )
            at_sb = tpool.tile([128, QPC], f32, tag="atsb")
            nc.scalar.copy(at_sb[:], at_ps[:])
            nc.tensor.matmul(
                pv_ps[:, b * QPC : (b + 1) * QPC],
                lhsT=vals_sb[(b, jt)][:],
                rhs=at_sb[:],
                start=(jt == 0),
                stop=(jt == nj - 1),
            )

        if PAIR_OUT:
            # stack 1/rowsum for the adjacent pair; output emitted when both done
            rp = rinvp[b // 2]
            nc.vector.reciprocal(rp[64 * (b % 2) : 64 * (b % 2) + 64, :], rowsum[:])
            tails_done.add(b)
            if (b ^ 1) in tails_done:
                p2 = b // 2
                pvb_sb = tpool.tile([D, 2 * QPC], f32, name=f"pvbp{p2}", tag="atsb")
                nc.vector.tensor_copy(
                    pvb_sb[:], pv_ps[:, 2 * QPC * p2 : 2 * QPC * (p2 + 1)]
                )
                o_ps = tps.tile([2 * QPC, H], f32, name=f"opsp{p2}", tag="atps")
                nc.tensor.matmul(
                    o_ps[:], lhsT=pvb_sb[:], rhs=wo_sb[:], start=True, stop=True
                )
                o_sb = opool.tile([2 * QPC, H], f32, name=f"osbp{p2}", tag="osb")
                nc.vector.tensor_scalar_mul(o_sb[:], o_ps[:], rp[:])
                nc.sync.dma_start(
                    out_d[2 * QPC * p2 : 2 * QPC * (p2 + 1), :], o_sb[:]
                )
        else:
            # output projection for this batch (1/rowsum folded into the copy)
            pvb_sb = tpool.tile([D, QPC], f32, name=f"pvb{b}", tag="atsb")
            nc.vector.tensor_copy(pvb_sb[:], pv_ps[:, b * QPC : (b + 1) * QPC])
            o_ps = tps.tile([QPC, H], f32, name=f"ops{b}", tag="atps")
            nc.tensor.matmul(
                o_ps[:], lhsT=pvb_sb[:], rhs=wo_sb[:], start=True, stop=True
            )
            o_sb = opool.tile([QPC, H], f32, name=f"osb{b}", tag="osb")
            nc.vector.tensor_scalar_mul(o_sb[:], o_ps[:], rinv[:])
            nc.sync.dma_start(out_d[b * QPC : (b + 1) * QPC, :], o_sb[:])

    tails_done = set()
    rinvp = {}
    if PAIR_OUT:
        for p2 in range(B // 2):
            t = stat.tile([2 * QPC, 1], f32, name=f"rinvp{p2}", tag=f"rinvp{p2}")
            rinvp[p2] = t

    sc_tiles = {}
    for b in order:
        sc_tiles[b] = scpool.tile([QPC, 512], f32, name=f"sc{b}", tag=f"sc{b}")
    # balance DVE (broadcast adds) against ACT (tanh) by giving ACT a few
    # fused bias-queries from the largest batch
    # measured on HW: ACT-bias queries disrupt the ACT stream more than their
    # nominal marginal cost -- keep everything on the chunk path
    nbias = {b: 0 for b in range(B)}

    for i, b in enumerate(order):
        emit_chunks(b, sc_tiles[b], last=(i == B - 1), nbias=nbias[b])
        if nbias[b]:
            emit_bias_queries(b, sc_tiles[b], QPC - nbias[b], nbias[b])
        if variant.startswith("full") and i > 0:
            emit_tail(order[i - 1], sc_tiles[order[i - 1]])
        if stage_cb is not None and i < B - 1:
            stage_cb()
    if variant.startswith("full"):
        emit_tail(order[-1], sc_tiles[order[-1]])


def _build_program(valid: tuple, iters: int = 1, variant: str = "full"):
    import concourse.bacc as bacc
    import concourse.mybir as mybir
    import concourse.tile as tile

    f32 = mybir.dt.float32
    fp16 = mybir.dt.float16
    AF = mybir.ActivationFunctionType
    AX = mybir.AxisListType

    nc = bacc.Bacc("TRN2", target_bir_lowering=False, debug=False)

    dram = (
        nc.dram_tensor("qT", [D, B * QPC], f32, kind="ExternalInput"),
        nc.dram_tensor("kT", [B * D, LK], f32, kind="ExternalInput"),
        nc.dram_tensor("vals", [B * LK, D], f32, kind="ExternalInput"),
        nc.dram_tensor("wq", [D, H], f32, kind="ExternalInput"),
        nc.dram_tensor("wk", [D, H], f32, kind="ExternalInput"),
        nc.dram_tensor("wo", [D, H], f32, kind="ExternalInput"),
        nc.dram_tensor("zmat", [H, 63], fp16, kind="ExternalInput"),
        nc.dram_tensor("ident", [128, 128], f32, kind="ExternalInput"),
        nc.dram_tensor("out", [B * QPC, H], f32, kind="ExternalOutput"),
    )

    njs = [max(1, math.ceil(v / 128)) for v in valid]

    with tile.TileContext(nc, pool_alloc_mode=POOL_ALLOC) as tc, ExitStack() as ctx:
        consts = ctx.enter_context(tc.tile_pool(name="consts", bufs=1))
        loads = ctx.enter_context(tc.tile_pool(name="loads", bufs=2))
        if iters == 1:
            _emit_body(nc, tc, ctx, consts, loads, valid, njs, dram, f32, fp16, AF, AX, variant)
        elif iters < 0:  # straight-line unrolled -iters times (bench sanity check)
            for _ in range(-iters):
                with ExitStack() as ictx:
                    _emit_body(nc, tc, ictx, consts, loads, valid, njs, dram, f32, fp16, AF, AX, variant)
        else:
            with tc.For_i(0, iters, 1, staggered_reset=True):
                with ExitStack() as ictx:
                    _emit_body(nc, tc, ictx, consts, loads, valid, njs, dram, f32, fp16, AF, AX, variant)

    nc.compile()
    return nc


class Runner:
    """Cached jitted shard_map over the 8 cores, reusable across calls."""

    def __init__(self, nc):
        import jax
        import concourse.mybir as mybir
        from concourse import bass2jax
        from jax.sharding import Mesh, PartitionSpec
        from jax.experimental.shard_map import shard_map

        bass2jax.install_neuronx_cc_hook()
        self.jax = jax

        partition_name = nc.partition_id_tensor.name if nc.partition_id_tensor else None
        in_names, out_names, out_avals, zero_outs = [], [], [], []
        for alloc in nc.m.functions[0].allocations:
            if not isinstance(alloc, mybir.MemoryLocationSet):
                continue
            name = alloc.memorylocations[0].name
            if alloc.kind == "ExternalInput":
                if name != partition_name:
                    in_names.append(name)
            elif alloc.kind == "ExternalOutput":
                out_names.append(name)
                shape = tuple(alloc.tensor_shape)
                dtype = mybir.dt.np(alloc.dtype)
                out_avals.append(jax.core.ShapedArray(shape, dtype))
                zero_outs.append(np.zeros(shape, dtype))
        self.in_names = in_names
        self.n_params = len(in_names)
        n_outs = len(out_avals)
        all_in_names = in_names + out_names
        if partition_name is not None:
            all_in_names = all_in_names + [partition_name]
        self.out_names = out_names
        self.out_avals = out_avals
        self.zero_outs = zero_outs

        def _body(*args):
            operands = list(args)
            if partition_name is not None:
                operands.append(bass2jax.partition_id_tensor())
            outs = bass2jax._bass_exec_p.bind(
                *operands,
                out_avals=tuple(out_avals),
                in_names=tuple(all_in_names),
                out_names=tuple(out_names),
                lowering_input_output_aliases=(),
                sim_require_finite=True,
                sim_require_nnan=True,
                nc=nc,
            )
            return tuple(outs)

        devices = jax.devices()[:NCORES]
        mesh = Mesh(np.asarray(devices), ("core",))
        n_all = self.n_params + n_outs
        self.fn = jax.jit(
            shard_map(
                _body,
                mesh=mesh,
                in_specs=(PartitionSpec("core"),) * n_all,
                out_specs=(PartitionSpec("core"),) * n_outs,
                check_rep=False,
            ),
            donate_argnums=tuple(range(self.n_params, n_all)),
            keep_unused=True,
        )

    def stage_inputs(self, in_maps):
        per_core = [[np.asarray(m[name]) for name in self.in_names] for m in in_maps]
        return [
            self.jax.device_put(
                np.concatenate([per_core[c][i] for c in range(NCORES)], axis=0)
            )
            for i in range(self.n_params)
        ]

    def fresh_zeros(self):
        return [
            self.jax.device_put(np.zeros((NCORES * z.shape[0], *z.shape[1:]), z.dtype))
            for z in self.zero_outs
        ]

    def run(self, staged_inputs):
        outs = self.fn(*staged_inputs, *self.fresh_zeros())
        self.jax.block_until_ready(outs)
        i = self.out_names.index("out")
        return [
            np.asarray(outs[i]).reshape(NCORES, *self.out_avals[i].shape)[c]
            for c in range(NCORES)
        ]


def _get_runner(valid: tuple, iters: int = 1, variant: str = "full"):
    key = (valid, iters, variant)
    if key not in _RUNNERS:
        _RUNNERS[key] = Runner(_build_program(valid, iters, variant))
    return _RUNNERS[key]


def make_in_maps(queries, keys, values, valid_seq_len, Wq, Wk, wv, Wo):
    queries = np.asarray(queries, np.float32)
    keys = np.asarray(keys, np.float32)
    values = np.asarray(values, np.float32)
    Wq = np.ascontiguousarray(np.asarray(Wq, np.float32))
    Wk = np.ascontiguousarray(np.asarray(Wk, np.float32))
    wv = np.asarray(wv, np.float32)
    Wo = np.ascontiguousarray(np.asarray(Wo, np.float32))

    qT_full = np.ascontiguousarray(queries.transpose(2, 0, 1))  # (D, B, Lq)
    kT = np.ascontiguousarray(keys.transpose(0, 2, 1)).reshape(B * D, LK)
    vals = np.ascontiguousarray(values.reshape(B * LK, D))
    zmat = np.zeros((H, 63), np.float16)
    zmat[:, 31] = wv.astype(np.float16)
    ident = np.eye(128, dtype=np.float32)

    in_maps = []
    for c in range(NCORES):
        qT_c = np.ascontiguousarray(
            qT_full[:, :, c * QPC : (c + 1) * QPC].reshape(D, B * QPC)
        )
        in_maps.append(
            dict(qT=qT_c, kT=kT, vals=vals, wq=Wq, wk=Wk, wo=Wo, zmat=zmat, ident=ident)
        )
    return in_maps


def assemble(outs):
    out = np.empty((B, LQ, H), np.float32)
    for c in range(NCORES):
        out[:, c * QPC : (c + 1) * QPC, :] = outs[c].reshape(B, QPC, H)
    return out


def kernel(queries, keys, values, valid_seq_len, Wq, Wk, wv, Wo):
    valid = tuple(int(v) for v in np.asarray(valid_seq_len))
    in_maps = make_in_maps(queries, keys, values, valid_seq_len, Wq, Wk, wv, Wo)
    last_err = None
    for attempt in range(3):
        try:
            runner = _get_runner(valid)
            return assemble(runner.run(runner.stage_inputs(in_maps)))
        except Exception as e:  # transient device wedge: rebuild the jit and retry
            last_err = e
            _RUNNERS.pop((valid, 1, "full"), None)
            import time as _time

            _time.sleep(2.0 * (attempt + 1))
    raise last_err

